# revision 25
# baseline (speedup 1.0000x reference)
"""Trainium2 Bass kernel for MultiHead GQA attention (B=2, S=2048, D=2048,
H=16 query heads, HKV=4 kv heads, DH=128, RoPE, mask, out-proj).

Sharding: token-parallel across 8 cores. Core c handles batch c//4 and 512
query rows of it (4 blocks of 128 rows). Each core projects K/V for its own
512-token quarter, all-gathers projected K/V across the 4 cores of its batch,
projects Q for its rows, runs attention + out-proj for its rows, and writes
its [512, 2048] slice. Host reassembles.

All matmuls run in bf16 with fp32 PSUM accumulation. Host pre-transposes /
pre-tiles every operand so each DMA is a contiguous [128, X] block and each
matmul consumes operands with the contraction dim on partitions.

Attention is computed transposed: scoresT[keys, q] = khT.T @ qhT per
128-key tile, exp on ScalarE (scale folded in), probs stored bf16, then
outT[dh, q] += v_tile.T @ probsT.

Row sums use "flipped" matmuls: each 128-column probs block is the
STATIONARY operand and a [128,1] ones vector is the MOVING operand, so the
PE streams a single column per block instead of re-streaming all probs
columns. The sums come out transposed ([q, 1], q on partitions) and
accumulate across key tiles into a shared PSUM bank (col h*4+block).
Normalization: reciprocal (DVE), PE-transpose back to row layout, bounce
through DRAM, broadcast-load, multiply into the unnormalized outputs.

Heads are processed in PAIRS sharing [128, 2, 512] PSUM tiles so exp /
mask-multiply / copies run as single batched instructions over both heads.

Mask handling (host-detected, compile-time mode):
  none   - mask has no zeros: no mask work at all.
  causal - mask is exactly tril: balanced interleaved q-blocks per core +
           suffix key-ranges (only ~62% of attention tiles computed), probs
           multiplied by the exact 0/1 mask tile.
  mask   - anything else: all tiles computed, probs multiplied by 0/1 mask.
"""

import math

import numpy as np
import ml_dtypes

import concourse.bass as bass
import concourse.mybir as mybir
import concourse.tile as tile
from concourse import bacc
from concourse.bass_utils import run_bass_kernel_spmd

F32 = mybir.dt.float32
BF16 = mybir.dt.bfloat16
BF = ml_dtypes.bfloat16

B, S, D = 2, 2048, 2048
H, G = 16, 4
HKV = H // G            # 4
DH = D // H             # 128
DKV = D // G            # 512 (kv projection width)
NCORES = 8
RPC = S // 4            # 512 rows per core
NQB = RPC // 128        # 4 q-blocks of 128 rows per core
NIC = D // 128          # 16 contraction chunks
NKC = S // 128          # 16 key tiles
SCALE = 1.0 / math.sqrt(DH)

_NC_CACHE: dict = {}

# set by callers (e.g. test.py) to capture a profile; results of the last run
TRACE = False
TRACE_CORES = None          # e.g. [0] or list(range(8))
LAST_RESULTS = None


def _n_list(mode: str) -> list[int]:
    """Moving-operand width (in q columns, suffix of the 512) per key tile."""
    if mode == "causal":
        # per key-tile kc, every core keeps exactly (4 - kc//4) of its 4
        # interleaved q-blocks {r, 7-r, 8+r, 15-r} (ascending order)
        return [128 * (4 - kc // 4) for kc in range(NKC)]
    return [512] * NKC


def _build(mode: str):
    mask_mul = mode != "none"
    n_list = _n_list(mode)

    # last key tile in which suffix position p (0=lowest of the 4 q-blocks)
    # still appears; the sums accumulation for that block stops there
    stop_kc = [max(kc for kc in range(NKC) if n_list[kc] // 128 >= 4 - p)
               for p in range(4)]

    nc = bacc.Bacc("TRN2", target_bir_lowering=False, debug=False,
                   num_devices=NCORES)

    # ---- I/O (host-prepared layouts; all contiguous-DMA friendly) ----
    wq = nc.declare_dram_parameter("wq", [NIC, 128, D], BF16, isOutput=False)
    qt = nc.declare_dram_parameter("qt", [128, NIC * RPC], BF16, isOutput=False)
    # k/v: only this core's 512-token quarter (projected here, all-gathered)
    kt = nc.declare_dram_parameter("kt", [128, NIC * 512], BF16, isOutput=False)
    vt = nc.declare_dram_parameter("vt", [4, 128, NIC * 128], BF16, isOutput=False)
    wk = nc.declare_dram_parameter("wk", [HKV, 128, NIC * 128], BF16, isOutput=False)
    wv = nc.declare_dram_parameter("wv", [128, NIC * DKV], BF16, isOutput=False)
    wo = nc.declare_dram_parameter("wo", [4, 128, H * 512], BF16, isOutput=False)
    cosq = nc.declare_dram_parameter("cosq", [128, RPC], BF16, isOutput=False)
    sinq = nc.declare_dram_parameter("sinq", [128, RPC], BF16, isOutput=False)
    # cos/sin for this core's own k-token quarter
    cosk = nc.declare_dram_parameter("cosk", [128, 512], BF16, isOutput=False)
    sink = nc.declare_dram_parameter("sink", [128, 512], BF16, isOutput=False)
    pswap = nc.declare_dram_parameter("pswap", [128, 128], BF16, isOutput=False)
    ident = nc.declare_dram_parameter("ident", [128, 128], F32, isOutput=False)
    if mode == "causal":
        # causal mask built on device from global row ids + partition index
        rowid = nc.declare_dram_parameter("rowid", [1, RPC], F32,
                                          isOutput=False)
        pidx = nc.declare_dram_parameter("pidx", [128, 1], F32, isOutput=False)
    elif mode == "mask":
        m01 = nc.declare_dram_parameter("m01", [128, NKC * RPC], BF16,
                                        isOutput=False)
    out = nc.declare_dram_parameter("out", [RPC, D], BF16, isOutput=True)

    with tile.TileContext(nc) as tc:
        with (
            tc.tile_pool(name="res", bufs=1) as res,          # resident
            tc.tile_pool(name="stream2m", bufs=2) as stream2m,  # 2MB blocks
            tc.tile_pool(name="stream1m", bufs=2) as stream1m,  # 1MB pairs
            tc.tile_pool(name="small", bufs=3) as small,
            tc.tile_pool(name="probs", bufs=4) as probsp,
            tc.tile_pool(name="bcast", bufs=2) as bcastp,
            tc.tile_pool(name="dram", bufs=1, space="DRAM") as dramp,
            tc.tile_pool(name="psmm", bufs=2 if mode == "causal" else 1,
                         space="PSUM") as psmm,
            tc.tile_pool(name="psacc", bufs=2, space="PSUM") as psacc,
            tc.tile_pool(name="pssum", bufs=1, space="PSUM") as pssum,
        ):
            # ---------------- resident tiles (DMAs staged per phase) -------
            # K-path first so the first matmul isn't stuck behind bulk loads
            kmov = stream2m.tile([128, NIC, 512], BF16, tag="s2m")
            nc.sync.dma_start(out=kmov, in_=kt[:, :].rearrange(
                "p (i m) -> p i m", i=NIC))
            coskq_t = res.tile([128, 512], BF16)
            nc.sync.dma_start(out=coskq_t, in_=cosk[:, :])
            sinkq_t = res.tile([128, 512], BF16)
            nc.sync.dma_start(out=sinkq_t, in_=sink[:, :])
            pswap_t = res.tile([128, 128], BF16)
            nc.sync.dma_start(out=pswap_t, in_=pswap[:, :])
            ident_t = res.tile([128, 128], F32)
            nc.sync.dma_start(out=ident_t, in_=ident[:, :])
            ones_t = res.tile([128, 1], BF16)
            nc.vector.memset(ones_t, 1.0)
            # allocated here (tag order: qts before outu_a), loaded later
            qts = res.tile([128, NIC, RPC], BF16)

            qhs = res.tile([128, H, RPC], BF16)     # rope'd q, [dh, h, rows]
            khs = res.tile([128, HKV, S], BF16)     # rope'd k, [dh, hk, keys]
            vhs = res.tile([128, 16, DKV], BF16)    # v heads, [tok%128, tokc, kv]
            # outu_a shares qts's slot: qts is dead once Q proj finishes.
            # split 12/4 so out-proj's early matmuls (h<12) don't dep-chain
            # behind the last normalization batch (h>=12).
            outu_a = res.tile([128, 12, RPC], BF16, tag="qts")
            outu_b = res.tile([128, 4, RPC], BF16)

            def outu(h):
                return outu_a[:, h, :] if h < 12 else outu_b[:, h - 12, :]

            # normalization batches (head ranges, 4-aligned)
            NB = [(0, 8), (8, 12), (12, 16)]
            rec_t_sb = [res.tile([32, 128], BF16, name=f"rect{g}",
                                 tag=f"rect{g}") for g in range(len(NB))]
            rec_dram = dramp.tile([4, 4 * RPC], BF16)   # row g: heads 4g..4g+3
            khs_own = res.tile([128, HKV, 512], BF16)
            vhs_own = res.tile([128, 4, DKV], BF16)
            kv_own = dramp.tile([2, 128, HKV, 512], BF16)
            kv_all = dramp.tile([4, 2, 128, HKV, 512], BF16)

            def rope_pair(dst2, x2_bf, cos_ap, sin_ap):
                """dst2[:, i, :] = x2[:, i, :]*cos + pswap(x2[:, i, :])*sin
                (signs baked into sin); x2/dst2 are [128, 2, 512]."""
                y0 = psacc.tile([128, 512], F32, tag="acc")
                nc.tensor.matmul(y0, pswap_t, x2_bf[:, 0, :],
                                 start=True, stop=True)
                y1 = psacc.tile([128, 512], F32, tag="acc")
                nc.tensor.matmul(y1, pswap_t, x2_bf[:, 1, :],
                                 start=True, stop=True)
                t1 = small.tile([128, 2, 512], BF16, tag="t1")
                c2 = cos_ap[:, :].unsqueeze(1).broadcast_to([128, 2, 512])
                nc.vector.tensor_tensor(out=t1[:, :, :], in0=x2_bf[:, :, :],
                                        in1=c2, op=mybir.AluOpType.mult)
                t2 = small.tile([128, 2, 512], BF16, tag="t2")
                nc.vector.tensor_mul(t2[:, 0, :], y0, sin_ap)
                nc.vector.tensor_mul(t2[:, 1, :], y1, sin_ap)
                nc.vector.tensor_add(dst2, t1, t2)

            # ------- Phase B: K/V proj for OWN 512-token quarter + RoPE -----
            # (first, so the all-gather overlaps the Q projection below)
            for hp in range(HKV // 2):          # kv head pairs
                wk_all = stream1m.tile([128, 2, NIC, 128], BF16, tag="s1")
                nc.sync.dma_start(out=wk_all[:, 0], in_=wk[2 * hp].rearrange(
                    "p (i m) -> p i m", i=NIC))
                nc.sync.dma_start(out=wk_all[:, 1],
                                  in_=wk[2 * hp + 1].rearrange(
                                      "p (i m) -> p i m", i=NIC))
                ps = psmm.tile([128, 2, 512], F32, tag="mm")
                for j in range(2):
                    for ic in range(NIC):
                        nc.tensor.matmul(ps[:, j, :], wk_all[:, j, ic, :],
                                         kmov[:, ic, :],
                                         start=(ic == 0), stop=(ic == NIC - 1))
                xk = small.tile([128, 2, 512], BF16, tag="xq")
                nc.scalar.copy(xk, ps)
                rope_pair(khs_own[:, 2 * hp:2 * hp + 2, :], xk,
                          coskq_t, sinkq_t)
            # stage the K half of the gather input as soon as K rope is done
            nc.sync.dma_start(out=kv_own[0], in_=khs_own)

            wvs = res.tile([128, NIC, DKV], BF16)
            nc.sync.dma_start(out=wvs, in_=wv[:, :].rearrange(
                "p (i n) -> p i n", i=NIC))
            for jp in range(2):            # own 128-token block pairs
                vmov = stream1m.tile([128, 2, NIC, 128], BF16, tag="s1")
                nc.sync.dma_start(out=vmov[:, 0], in_=vt[2 * jp].rearrange(
                    "p (i m) -> p i m", i=NIC))
                nc.sync.dma_start(out=vmov[:, 1],
                                  in_=vt[2 * jp + 1].rearrange(
                                      "p (i m) -> p i m", i=NIC))
                ps = psmm.tile([128, 2, 512], F32, tag="mm")
                for j in range(2):
                    for ic in range(NIC):
                        nc.tensor.matmul(ps[:, j, :], vmov[:, j, ic, :],
                                         wvs[:, ic, :],
                                         start=(ic == 0), stop=(ic == NIC - 1))
                nc.vector.tensor_copy(vhs_own[:, 2 * jp:2 * jp + 2, :], ps)

            # ---- all-gather projected K/V across the 4 cores of the batch --
            nc.sync.dma_start(out=kv_own[1], in_=vhs_own)
            nc.gpsimd.collective_compute(
                "AllGather", mybir.AluOpType.bypass,
                replica_groups=[[0, 1, 2, 3], [4, 5, 6, 7]],
                ins=[kv_own[:, :, :, :]], outs=[kv_all[:, :, :, :, :]])
            # K quarters first: scores need them before the AV matmuls need V
            for r in range(4):
                nc.sync.dma_start(out=khs[:, :, r * 512:(r + 1) * 512],
                                  in_=kv_all[r, 0])
            for r in range(4):
                nc.sync.dma_start(out=vhs[:, 4 * r:4 * r + 4, :],
                                  in_=kv_all[r, 1])

            # ---------------- Phase A: Q-proj + RoPE (head pairs) ----------
            nc.sync.dma_start(out=qts, in_=qt[:, :].rearrange(
                "p (i m) -> p i m", i=NIC))
            cosq_t = res.tile([128, RPC], BF16)
            nc.sync.dma_start(out=cosq_t, in_=cosq[:, :])
            sinq_t = res.tile([128, RPC], BF16)
            nc.sync.dma_start(out=sinq_t, in_=sinq[:, :])
            if mode == "causal":
                # build the 0/1 causal mask on device:
                # m01s[p, kc, q] = (rowid[q] - 128*kc) >= p
                rowb = res.tile([128, RPC], F32)
                nc.sync.dma_start(out=rowb,
                                  in_=rowid[0:1, :].to_broadcast([128, RPC]))
                pidx_t = res.tile([128, 1], F32)
                nc.sync.dma_start(out=pidx_t, in_=pidx[:, :])
                m01s = res.tile([128, NKC, RPC], BF16)
                for kc in range(NKC):
                    nc.vector.tensor_scalar(
                        out=m01s[:, kc, :], in0=rowb,
                        scalar1=float(128 * kc), scalar2=pidx_t,
                        op0=mybir.AluOpType.subtract,
                        op1=mybir.AluOpType.is_ge)
            elif mode == "mask":
                m01s = res.tile([128, NKC, RPC], BF16)
                nc.sync.dma_start(out=m01s, in_=m01[:, :].rearrange(
                    "p (k m) -> p k m", k=NKC))
            for p in range(H // 2):
                wq_all = stream1m.tile([128, 2, NIC, 128], BF16, tag="s1")
                nc.sync.dma_start(out=wq_all[:, 0], in_=wq[2 * p].rearrange(
                    "p (i m) -> p i m", i=NIC))
                nc.sync.dma_start(out=wq_all[:, 1],
                                  in_=wq[2 * p + 1].rearrange(
                                      "p (i m) -> p i m", i=NIC))
                ps = psmm.tile([128, 2, 512], F32, tag="mm")
                for j in range(2):
                    for ic in range(NIC):
                        nc.tensor.matmul(ps[:, j, :], wq_all[:, j, ic, :],
                                         qts[:, ic, :],
                                         start=(ic == 0), stop=(ic == NIC - 1))
                xq = small.tile([128, 2, 512], BF16, tag="xq")
                nc.scalar.copy(xq, ps)
                rope_pair(qhs[:, 2 * p:2 * p + 2, :], xq, cosq_t, sinq_t)

            # ---------------- Phase C: attention per head pair -------------
            # Transposed row-sums (q on partitions). PSUM accumulation
            # contexts are per-bank, so interleaved multi-instruction groups
            # in one bank corrupt each other: every flip matmul is single-shot
            # into its own column, then a DVE free-dim reduce folds the
            # per-key-tile columns.
            #   slot(pos, h, kc) = SLOT_BASE[pos] + h*(stop_kc[pos]+1) + kc
            # finals at FIN0 + 4h+pos; transpose scratch at TSC0..TSC0+128.
            cnt = [stop_kc[p] + 1 for p in range(4)]
            SLOT_BASE = [0]
            for p in range(3):
                SLOT_BASE.append(SLOT_BASE[-1] + H * cnt[p])
            FIN0 = SLOT_BASE[-1] + H * cnt[3]
            TSC0 = FIN0 + 64
            nsum = 1024 if mode == "causal" else 1536
            assert TSC0 + 128 <= nsum
            sums_t = pssum.tile([128, nsum], F32, tag="sum")

            def normalize_batch(g):
                """reduce + reciprocal + transpose + DRAM bounce + broadcast
                + in-place normalize for the heads of batch g."""
                a, bnd = NB[g]
                m = 4 * (bnd - a)
                fin = sums_t[:, FIN0 + 4 * a:FIN0 + 4 * bnd]
                for p in range(4):
                    sl = sums_t[:, SLOT_BASE[p] + a * cnt[p]:
                                SLOT_BASE[p] + bnd * cnt[p]]
                    nc.vector.tensor_reduce(
                        out=fin.rearrange("q (h f) -> q h f", f=4)[:, :, p],
                        in_=sl.rearrange("q (h c) -> q h c", c=cnt[p]),
                        axis=mybir.AxisListType.X, op=mybir.AluOpType.add)
                rec_sb = small.tile([128, 32], F32, tag="rec", bufs=2)
                nc.vector.reciprocal(rec_sb[:, :m], fin)
                # PE transpose -> [m, 128] into the scratch region
                nc.tensor.matmul(sums_t[0:m, TSC0:TSC0 + 128],
                                 rec_sb[:, :m], ident_t,
                                 is_transpose=True, skip_group_check=True)
                nc.vector.tensor_copy(rec_t_sb[g][:m, :],
                                      sums_t[0:m, TSC0:TSC0 + 128])
                nc.sync.dma_start(
                    out=rec_dram[a // 4:bnd // 4, :],
                    in_=rec_t_sb[g][:m, :])
                for h4 in range(a, bnd, 4):
                    recb = bcastp.tile([128, 4 * RPC], BF16, tag="bc")
                    nc.sync.dma_start(
                        out=recb,
                        in_=rec_dram[h4 // 4:h4 // 4 + 1, :]
                        .to_broadcast([128, 4 * RPC]))
                    if h4 < 12:
                        lo_t = outu_a[:, h4:h4 + 4, :]
                    else:
                        lo_t = outu_b[:, 0:4, :]
                    lo2 = lo_t.rearrange("p h n -> p (h n)")
                    nc.vector.tensor_mul(lo2, lo2, recb)

            for p in range(H // 2):
                h0 = 2 * p
                hk = h0 // G
                ps_o0 = psacc.tile([128, 512], F32, tag="acc")
                ps_o1 = psacc.tile([128, 512], F32, tag="acc")
                for kc in range(NKC):
                    n = n_list[kc]
                    nb = n // 128
                    lo = RPC - n          # suffix columns
                    ps_sc = psmm.tile([128, 2, 512], F32, tag="mm")
                    nc.tensor.matmul(
                        ps_sc[:, 0, :n],
                        khs[:, hk, kc * 128:(kc + 1) * 128],
                        qhs[:, h0, lo:],
                        start=True, stop=True, skip_group_check=True)
                    nc.tensor.matmul(
                        ps_sc[:, 1, :n],
                        khs[:, hk, kc * 128:(kc + 1) * 128],
                        qhs[:, h0 + 1, lo:],
                        start=True, stop=True, skip_group_check=True)
                    probs = probsp.tile([128, 2, 512], BF16, tag="pr")
                    nc.scalar.activation(
                        probs[:, :, :n], ps_sc[:, :, :n],
                        mybir.ActivationFunctionType.Exp, scale=SCALE)
                    if mask_mul:
                        # causal: only the lowest <=2 blocks of the suffix can
                        # contain masked entries (padding + diagonal); above
                        # the diagonal every block is fully kept.
                        nm = min(256, n) if mode == "causal" else n
                        mb = m01s[:, kc:kc + 1, lo:lo + nm] \
                            .broadcast_to([128, 2, nm])
                        nc.vector.tensor_tensor(
                            out=probs[:, :, :nm], in0=probs[:, :, :nm],
                            in1=mb, op=mybir.AluOpType.mult)
                    first = kc == 0
                    last = kc == NKC - 1
                    nc.tensor.matmul(
                        ps_o0[:, lo:],
                        vhs[:, kc, hk * 128:(hk + 1) * 128],
                        probs[:, 0, :n],
                        start=first, stop=last, skip_group_check=True)
                    nc.tensor.matmul(
                        ps_o1[:, lo:],
                        vhs[:, kc, hk * 128:(hk + 1) * 128],
                        probs[:, 1, :n],
                        start=first, stop=last, skip_group_check=True)
                    # flipped row-sums: probs block stationary, ones moving,
                    # single-shot into a dedicated column per (h, pos, kc)
                    for j in range(nb):
                        pos = 4 - nb + j
                        for i in range(2):
                            col = SLOT_BASE[pos] + (h0 + i) * cnt[pos] + kc
                            nc.tensor.matmul(
                                sums_t[:, col:col + 1],
                                probs[:, i, j * 128:(j + 1) * 128],
                                ones_t,
                                start=True, stop=True,
                                skip_group_check=True)
                nc.vector.tensor_copy(outu(h0), ps_o0)
                nc.vector.tensor_copy(outu(h0 + 1), ps_o1)
                if h0 + 2 == 8:
                    normalize_batch(0)
                elif h0 + 2 == 12:
                    normalize_batch(1)
            normalize_batch(2)

            # ---------------- Phase D: out-projection ----------------
            for oc in range(4):
                wo_all = stream2m.tile([128, H, 512], BF16, tag="s2m")
                nc.sync.dma_start(out=wo_all, in_=wo[oc].rearrange(
                    "p (h m) -> p h m", h=H))
                for qcp in range(NQB // 2):
                    ps_f = psmm.tile([128, 2, 512], F32, tag="mm")
                    for j in range(2):
                        qc = 2 * qcp + j
                        for h in range(H):
                            lh = outu_a[:, h, qc * 128:(qc + 1) * 128] \
                                if h < 12 else \
                                outu_b[:, h - 12, qc * 128:(qc + 1) * 128]
                            nc.tensor.matmul(
                                ps_f[:, j, :], lh, wo_all[:, h, :],
                                start=(h == 0), stop=(h == H - 1))
                    fin = small.tile([128, 2, 512], BF16, tag="fin", bufs=2)
                    nc.scalar.copy(fin, ps_f)
                    for j in range(2):
                        qc = 2 * qcp + j
                        nc.sync.dma_start(
                            out=out[qc * 128:(qc + 1) * 128,
                                    oc * 512:(oc + 1) * 512],
                            in_=fin[:, j, :])

    nc.compile()
    return nc


def _get_nc(mode: str):
    if mode not in _NC_CACHE:
        _NC_CACHE[mode] = _build(mode)
    return _NC_CACHE[mode]


def _core_rows(mode: str, r: int) -> np.ndarray:
    """Global (within-batch) q-row indices owned by quarter r, ascending."""
    if mode == "causal":
        blocks = sorted([r, 7 - r, 8 + r, 15 - r])
    else:
        blocks = [4 * r, 4 * r + 1, 4 * r + 2, 4 * r + 3]
    return np.concatenate([np.arange(b * 128, (b + 1) * 128) for b in blocks])


def kernel(q, k, v, mask, freqs, W_q, W_k, W_v, W_o):
    q = np.asarray(q, dtype=np.float32)
    k = np.asarray(k, dtype=np.float32)
    v = np.asarray(v, dtype=np.float32)
    mask = np.asarray(mask, dtype=np.float32)
    freqs = np.asarray(freqs, dtype=np.float32)
    W_q = np.asarray(W_q, dtype=np.float32)
    W_k = np.asarray(W_k, dtype=np.float32)
    W_v = np.asarray(W_v, dtype=np.float32)
    W_o = np.asarray(W_o, dtype=np.float32)

    # ---- mask mode detection ----
    nz = mask != 0
    if nz.all():
        mode = "none"
    else:
        tril = np.tril(np.ones((S, S), dtype=bool))
        mode = "causal" if all(np.array_equal(nz[b], tril) for b in range(B)) \
            else "mask"

    # ---- shared host precomputation ----
    c_full = np.cos(freqs)                      # [S, 64]
    s_full = np.sin(freqs)
    sgn = np.tile(np.array([-1.0, 1.0], np.float32), DH // 2)  # [-,+,-,+...]
    cosk_h = np.repeat(c_full, 2, axis=1).T.astype(BF)          # [128, S]
    sink_h = (np.repeat(s_full, 2, axis=1) * sgn).T.astype(BF)

    psw = np.zeros((128, 128), np.float32)
    idx = np.arange(128)
    psw[idx, idx ^ 1] = 1.0
    psw = psw.astype(BF)
    identity = np.eye(128, dtype=np.float32)

    # weight layouts
    # wq[oc, p, i*128+m] = W_q[oc*128+m, i*128+p]
    wq_h = np.ascontiguousarray(
        W_q.reshape(H, 128, NIC, 128).transpose(0, 3, 2, 1)
        .reshape(H, 128, D)).astype(BF)
    # wk[hk, p, i*128+m] = W_k[hk*128+m, i*128+p]
    wk_h = np.ascontiguousarray(
        W_k.reshape(HKV, 128, NIC, 128).transpose(0, 3, 2, 1)
        .reshape(HKV, 128, D)).astype(BF)
    # wv[p, i*512+n] = W_v[n, i*128+p]
    wv_h = np.ascontiguousarray(
        W_v.reshape(DKV, NIC, 128).transpose(2, 1, 0).reshape(128, NIC * DKV)
    ).astype(BF)
    # wo[oc, p, h*512+m] = W_o[oc*512+m, h*128+p]
    wo_h = np.ascontiguousarray(
        W_o.reshape(4, 512, H, 128).transpose(0, 3, 2, 1).reshape(4, 128, -1)
    ).astype(BF)

    # k/v: each core only gets its own 512-token quarter (gathered on device)
    # kt[p, i*512+t] = k[b, tq*512+t, i*128+p] for quarter tq
    kt_b = []   # [B][4] quarters
    vt_b = []
    for b in range(B):
        kt_b.append([np.ascontiguousarray(
            k[b, tq * 512:(tq + 1) * 512].reshape(512, NIC, 128)
            .transpose(2, 1, 0).reshape(128, NIC * 512)).astype(BF)
            for tq in range(4)])
        # vt[j, p, i*128+t] = v[b, tq*512 + j*128+t, i*128+p]
        vt_b.append([np.ascontiguousarray(
            v[b, tq * 512:(tq + 1) * 512].reshape(4, 128, NIC, 128)
            .transpose(0, 3, 2, 1).reshape(4, 128, NIC * 128)).astype(BF)
            for tq in range(4)])

    in_maps = []
    rows_all = []
    for c in range(NCORES):
        b, r = divmod(c, 4)
        rows = _core_rows(mode, r)
        rows_all.append((b, rows))
        # qt[p, i*512+t] = q[b, rows[t], i*128+p]
        qsl = q[b][rows]                       # [512, D]
        qt_h = np.ascontiguousarray(
            qsl.reshape(RPC, NIC, 128).transpose(2, 1, 0).reshape(128, -1)
        ).astype(BF)
        cq = np.repeat(c_full[rows], 2, axis=1).T.astype(BF)      # [128, 512]
        sq = (np.repeat(s_full[rows], 2, axis=1) * sgn).T.astype(BF)
        im = {
            "wq": wq_h, "qt": qt_h, "kt": kt_b[b][r], "vt": vt_b[b][r],
            "wk": wk_h, "wv": wv_h, "wo": wo_h,
            "cosq": cq, "sinq": sq,
            "cosk": np.ascontiguousarray(cosk_h[:, r * 512:(r + 1) * 512]),
            "sink": np.ascontiguousarray(sink_h[:, r * 512:(r + 1) * 512]),
            "pswap": psw, "ident": identity,
        }
        if mode == "causal":
            im["rowid"] = rows.astype(np.float32)[None, :]
            im["pidx"] = np.arange(128, dtype=np.float32)[:, None]
        elif mode == "mask":
            # m01[p, kc*512+m] = (mask[b, rows[m], kc*128+p] != 0)
            msl = nz[b][rows]                  # [512, S] bool
            m01_h = np.ascontiguousarray(
                msl.T.reshape(NKC, 128, RPC).transpose(1, 0, 2)
                .reshape(128, -1)).astype(BF)
            im["m01"] = m01_h
        in_maps.append(im)

    nc = _get_nc(mode)
    kwargs = {}
    if TRACE:
        kwargs["trace"] = True
        if TRACE_CORES:
            kwargs["trace_cores"] = list(TRACE_CORES)
    results = run_bass_kernel_spmd(nc, in_maps, core_ids=list(range(NCORES)),
                                   **kwargs)
    global LAST_RESULTS
    LAST_RESULTS = results

    full = np.empty((B, S, D), np.float32)
    for c in range(NCORES):
        b, rows = rows_all[c]
        full[b, rows] = results.results[c]["out"].astype(np.float32)
    return full
